# revision 29
# baseline (speedup 1.0000x reference)
"""FourierFT fused kernel for Trainium2 (8 NeuronCores, SPMD data-parallel).

Computes h = x @ W_base^T + b_base + x @ Delta_W where
Delta_W = real(ifft2(scatter(c, E))) * ALPHA. The rank-200 Delta_W is folded
into the weight on the host (two [4096,100]x[100,4096] sgemms), so the device
runs a dense GEMM h^T = W_eff-contracted-with-x^T plus bias.

The GEMM is PE-bound (bf16 floor 437us/core; DMA measured 312 GB/s/core has
3x headroom), so the contraction is split by precision: the first KF=768 of
4096 k-values run as fp8(e4m3) DoubleRow matmuls (2 k-tiles per instruction,
2x throughput), the remaining 26 k-tiles in bf16. The fp8 fraction is capped
by accuracy: measured on the real inputs, 6/32 fp8 tiles give 1.6e-2 max rel
error vs the 2e-2 budget (8/32 would be 2.0e-2 - no margin).

Quantization: x8 = e4m3(x * sx), sx = 240/max|x| global; W8 col l scaled by
g[l] = 240/max|W_col|; the bf16 part of W is pre-scaled by sx*g[l] so both
parts accumulate in one PSUM in the same scaled units; the drain activation
applies the per-partition (per-l) scale 1/(sx*g[l]) and bias in one pass,
storing fp16 (halves output traffic; +6e-5 error).

Device layout: each core owns a 1024-row slice of x (flattened [8192,4096]),
pre-transposed k-major on the host. Output is produced as h^T tiles ([l,s]).
Schedule: the first ~30us is briefly DMA-bound (x + first weight chunks all
wanted early vs the 312 GB/s/core fabric cap), so the prefix issues pieces
in consumption order: bulk x and lo0 weight pieces interleaved on Sync
(ahead of main-loop weight prefetch so it cannot starve x), x8/bias/lo1
weights/outputs on Activation; weight chunks prefetch two output-chunks
ahead (DMA-complete semaphores propagate ~900ns, so JIT prefetch stalls).
Warmup matmuls ramp the PE clock before real data lands. lo0/lo1 are split
into two visits (fp8 + first weight chunk each, then second chunks), holding
all 8 PSUM banks, which defers 1.66MB of weight DMA out of the DMA-bound
prefix window. The last output chunk runs bank-major so three of its four
PSUM drains hide under matmuls. Steady state issues one matmul per 216ns
with zero gaps; measured 422-428us = 401us issue floor + ~7.5us framework
preamble (before any DMA can issue) + ~6us first-data ramp + ~11.5us fixed
framework teardown.
"""

import sys

if "/opt/trn_rl_repo" not in sys.path:
    sys.path.insert(0, "/opt/trn_rl_repo")

import numpy as np
import ml_dtypes

import concourse.bass as bass  # noqa: F401  (registers AP machinery)
import concourse.mybir as mybir
import concourse.tile as tile
from concourse import bacc, bass_utils

D1 = 4096
D2 = 4096
ALPHA = 300.0
NCORES = 8
S_TOTAL = 4 * 2048
S = S_TOTAL // NCORES  # 1024 rows per core
KT = D1 // 128  # 32 k-tiles
NLO = 16  # output column chunks of 256

NF8 = 3  # fp8 DoubleRow pairs per output chunk (2 k-tiles each)
KF = NF8 * 256  # k-values covered by fp8
KTB = KT - 2 * NF8  # bf16 k-tiles (26)
NWCH = 2  # bf16 weight chunks per lo
KPCB = KTB // NWCH  # 13 k-tiles per bf16 weight chunk
NXCH = KTB // 2  # bf16 x chunks (pairs of k-tiles)

F32 = mybir.dt.float32
F16 = mybir.dt.float16
BF16 = mybir.dt.bfloat16
FP8 = mybir.dt.float8e4
IDENT = mybir.ActivationFunctionType.Identity
DR = mybir.MatmulPerfMode.DoubleRow
BF = ml_dtypes.bfloat16
F8 = ml_dtypes.float8_e4m3

_CACHE = {}


def _build_nc():
    nc = bacc.Bacc("TRN2", target_bir_lowering=False, debug=False)

    x8_d = nc.dram_tensor("x8c", [128, NF8, 2, S], FP8, kind="ExternalInput").ap()
    xb_d = nc.dram_tensor("xbc", [NXCH, 128, 2, S], BF16, kind="ExternalInput").ap()
    w8_d = nc.dram_tensor(
        "w8b", [NLO, 128, NF8, 2, 256], FP8, kind="ExternalInput"
    ).ap()
    wb_d = nc.dram_tensor(
        "wbb", [NLO, 128, NWCH, KPCB, 256], BF16, kind="ExternalInput"
    ).ap()
    bias_d = nc.dram_tensor("biasc", [128, 32], F32, kind="ExternalInput").ap()
    scal_d = nc.dram_tensor("scalc", [128, 32], F32, kind="ExternalInput").ap()
    ht_d = nc.dram_tensor("ht", [D2, S], F16, kind="ExternalOutput").ap()

    with tile.TileContext(nc) as tc:
        with (
            tc.tile_pool(name="resident", bufs=1) as rpool,
            tc.tile_pool(name="wstream", bufs=9) as wpool,
            tc.tile_pool(name="wmerged", bufs=3) as wmpool,
            tc.tile_pool(name="outstage", bufs=3) as opool,
            tc.tile_pool(name="psum", bufs=8, space="PSUM") as ppool,
        ):
            x8_sb = rpool.tile([128, NF8, 2, S], FP8, tag="x8")
            xb_sb = rpool.tile([128, KTB, S], BF16, tag="xb")
            bias_sb = rpool.tile([128, 32], F32, tag="bias")
            scal_sb = rpool.tile([128, 32], F32, tag="scal")
            warm_sb = rpool.tile([128, 256], BF16, tag="warm")
            nc.gpsimd.memset(warm_sb[:], 0.0)

            def new_banks(lo):
                return [
                    [ppool.tile([128, 512], F32, tag="pm",
                                name=f"pms_{lo}_{j}_{h}")
                     for h in range(2)]
                    for j in range(2)
                ]

            def mm_f8(pms, w8t, p):
                for j in range(2):
                    lhsT = w8t[:, p, :, j * 128 : (j + 1) * 128]
                    for h in range(2):
                        nc.tensor.matmul(
                            pms[j][h],
                            lhsT,
                            x8_sb[:, p, :, h * 512 : (h + 1) * 512],
                            start=(p == 0),
                            stop=False,
                            perf_mode=DR,
                        )

            def mm_bf(pms, wbt, t):
                q = t % KPCB
                for j in range(2):
                    lhsT = wbt[:, q, j * 128 : (j + 1) * 128]
                    for h in range(2):
                        nc.tensor.matmul(
                            pms[j][h],
                            lhsT,
                            xb_sb[:, t, h * 512 : (h + 1) * 512],
                            start=False,
                            stop=(t == KTB - 1),
                        )

            drain_ots = {}

            def drain_part(pms, lo, part):
                # one ACTIVATE (plus the output DMA when a j-pair completes);
                # called at spaced points in the next chunk so the PSUM-read
                # bursts don't cluster against the PE's PSUM writes
                j, h = part // 2, part % 2
                lsub = lo * 2 + j
                if h == 0:
                    drain_ots[j] = opool.tile([128, S], F16, tag="ot",
                                              name=f"ot_{lo}_{j}")
                ot = drain_ots[j]
                nc.scalar.activation(
                    ot[:, h * 512 : (h + 1) * 512],
                    pms[j][h],
                    IDENT,
                    bias=bias_sb[:, lsub : lsub + 1],
                    scale=scal_sb[:, lsub : lsub + 1],
                )
                if h == 1:
                    nc.scalar.dma_start(
                        ht_d[lsub * 128 : (lsub + 1) * 128, :], ot
                    )

            def drain(pms, lo):
                for part in range(4):
                    drain_part(pms, lo, part)

            def load_w(lo):
                # two 832KB DMAs per lo, not one 1.66MB: per-DMA-engine
                # throughput is ~22.5 GB/s, so weight-stream rate comes from
                # DMA parallelism (a single merged DMA starves the PE, +35us)
                w8t = wpool.tile([128, NF8, 2, 256], FP8, tag="w8",
                                 name=f"w8_{lo}")
                nc.sync.dma_start(w8t, w8_d[lo])
                wbmt = wmpool.tile([128, NWCH, KPCB, 256], BF16, tag="wbm",
                                   name=f"wbm_{lo}")
                for cb in range(NWCH):
                    nc.sync.dma_start(wbmt[:, cb], wb_d[lo][:, cb])
                return w8t, [wbmt[:, 0], wbmt[:, 1]]

            # ---- prefix: consumption-ordered streams + PE clock warmup.
            # The first ~30us is briefly DMA-bound (x + lo0/lo1 weights all
            # wanted early), so pieces are issued on the Sync queue in the
            # order the PE consumes them; x8 goes on the Activation queue
            # in pair-sized pieces so the first DR matmul starts ASAP.
            for p in range(NF8):
                nc.scalar.dma_start(
                    x8_sb[:, p : p + 1, :, :], x8_d[:, p : p + 1, :, :]
                )
            nc.scalar.dma_start(bias_sb[:], bias_d[:])
            nc.scalar.dma_start(scal_sb[:], scal_d[:])
            w8t0 = wpool.tile([128, NF8, 2, 256], FP8, tag="w8", name="w8_0")
            nc.sync.dma_start(w8t0, w8_d[0])
            wbts0 = [
                wpool.tile([128, KPCB, 256], BF16, tag="wb", name=f"wb_0_{cb}")
                for cb in range(NWCH)
            ]

            def wb0_piece(cb, a, bnd):
                nc.sync.dma_start(
                    wbts0[cb][:, a:bnd, :], wb_d[0][:, cb, a:bnd, :]
                )

            def xb_chunk(c, eng):
                eng.dma_start(xb_sb[:, 2 * c : 2 * c + 2, :], xb_d[c])

            # lo1 weights ride the otherwise-idle Activation queue; bulk x
            # and lo0 weight pieces interleave by need-time on Sync, ahead
            # of the main-loop weight prefetch so it cannot starve x. The
            # second weight chunks (wb*[1]) queue after the x stream — they
            # are not consumed until the second visits at t~38/49us.
            w8t1 = wpool.tile([128, NF8, 2, 256], FP8, tag="w8", name="w8_1")
            nc.scalar.dma_start(w8t1, w8_d[1])
            wbts1 = [
                wpool.tile([128, KPCB, 256], BF16, tag="wb", name=f"wb_1_{cb}")
                for cb in range(NWCH)
            ]
            nc.scalar.dma_start(wbts1[0], wb_d[1][:, 0])
            wb0_piece(0, 0, 2)
            xb_chunk(0, nc.sync)
            wb0_piece(0, 2, 5)
            xb_chunk(1, nc.sync)
            wb0_piece(0, 5, 9)
            xb_chunk(2, nc.sync)
            wb0_piece(0, 9, 13)
            xb_chunk(3, nc.sync)
            xb_chunk(4, nc.sync)
            xb_chunk(5, nc.sync)
            w_tiles = {0: (w8t0, wbts0), 1: (w8t1, wbts1)}
            for c in range(6, NXCH):
                xb_chunk(c, nc.sync if c % 2 == 0 else nc.scalar)
            wb0_piece(1, 0, 13)
            nc.scalar.dma_start(wbts1[1], wb_d[1][:, 1])

            # warm the PE clock off warm_sb (memset lands ~7.2us, before any
            # DMA): enough back-to-back matmuls to bridge until real data
            pms0 = new_banks(0)
            for _ in range(14):
                nc.tensor.matmul(
                    pms0[0][0][:, 0:256],
                    warm_sb[:, 0:128],
                    warm_sb[:, 0:256],
                    start=True,
                    stop=False,
                    skip_group_check=True,
                )

            # ---- lo0/lo1 split into two visits: both run their fp8 pairs +
            # first bf16 weight chunk before either touches its second chunk,
            # deferring 1.66MB of weight DMA out of the DMA-bound prefix
            # window (the early PE gaps were exactly this stream deficit).
            # Holds both bank sets: 4 + 4 = all 8 PSUM banks.
            w8t0, wbts0 = w_tiles.pop(0)
            w8t1, wbts1 = w_tiles.pop(1)
            pms1 = new_banks(1)
            for p in range(NF8):
                mm_f8(pms0, w8t0, p)
            for t in range(KPCB):
                mm_bf(pms0, wbts0[0], t)
            for p in range(NF8):
                mm_f8(pms1, w8t1, p)
            for t in range(KPCB):
                mm_bf(pms1, wbts1[0], t)
            w_tiles[2] = load_w(2)
            for t in range(KPCB, KTB):
                mm_bf(pms0, wbts0[1], t)
            w_tiles[3] = load_w(3)
            for t in range(KPCB, KTB):
                mm_bf(pms1, wbts1[1], t)
            drain(pms0, 0)

            # ---- main loop (last chunk handled bank-major below)
            prev = pms1
            prev_lo = 1
            for lo in range(2, NLO - 1):
                for ahead in (lo + 1, lo + 2):
                    if ahead < NLO and ahead not in w_tiles:
                        w_tiles[ahead] = load_w(ahead)
                w8t, wbts = w_tiles.pop(lo)
                pms = new_banks(lo)
                for p in range(NF8):
                    mm_f8(pms, w8t, p)
                if prev is not None:
                    drain_part(prev, prev_lo, 0)
                for t in range(KTB):
                    mm_bf(pms, wbts[t // KPCB], t)
                    if prev is not None and t in (5, 11, 17):
                        drain_part(prev, prev_lo, 1 + (t - 5) // 6)
                prev = pms
                prev_lo = lo

            # ---- last chunk: bank-major so three of the four PSUM drains
            # (and the first output DMA) hide under remaining matmuls
            lo = NLO - 1
            w8t, wbts = w_tiles.pop(lo)
            pms = new_banks(lo)
            ots = {}
            for idx, (j, h) in enumerate([(0, 0), (0, 1), (1, 0), (1, 1)]):
                for p in range(NF8):
                    nc.tensor.matmul(
                        pms[j][h],
                        w8t[:, p, :, j * 128 : (j + 1) * 128],
                        x8_sb[:, p, :, h * 512 : (h + 1) * 512],
                        start=(p == 0),
                        stop=False,
                        perf_mode=DR,
                    )
                if idx == 0:
                    drain(prev, prev_lo)
                for t in range(KTB):
                    q = t % KPCB
                    nc.tensor.matmul(
                        pms[j][h],
                        wbts[t // KPCB][:, q, j * 128 : (j + 1) * 128],
                        xb_sb[:, t, h * 512 : (h + 1) * 512],
                        start=False,
                        stop=(t == KTB - 1),
                    )
                if idx >= 1:
                    pj, ph = [(0, 0), (0, 1), (1, 0)][idx - 1]
                    lsub = lo * 2 + pj
                    if ph == 0:
                        ots[pj] = opool.tile([128, S], F16, tag="ot",
                                             name=f"otf{pj}")
                    nc.scalar.activation(
                        ots[pj][:, ph * 512 : (ph + 1) * 512],
                        pms[pj][ph],
                        IDENT,
                        bias=bias_sb[:, lsub : lsub + 1],
                        scale=scal_sb[:, lsub : lsub + 1],
                    )
                    if ph == 1:
                        nc.scalar.dma_start(
                            ht_d[lsub * 128 : (lsub + 1) * 128, :], ots[pj]
                        )
            lsub = lo * 2 + 1
            nc.scalar.activation(
                ots[1][:, 512:1024],
                pms[1][1],
                IDENT,
                bias=bias_sb[:, lsub : lsub + 1],
                scale=scal_sb[:, lsub : lsub + 1],
            )
            nc.scalar.dma_start(ht_d[lsub * 128 : (lsub + 1) * 128, :], ots[1])

    nc.compile()
    return nc


def _host_prep(x, c, E, W_base, b_base):
    """Fold Delta_W into W, quantize, shard + lay out inputs."""
    x2d = np.ascontiguousarray(
        np.asarray(x, dtype=np.float32).reshape(S_TOTAL, D1)
    )
    W = np.asarray(W_base, dtype=np.float32)
    b = np.asarray(b_base, dtype=np.float32)
    c32 = np.asarray(c, dtype=np.float32)
    u = np.asarray(E[0]).astype(np.int64)
    v = np.asarray(E[1]).astype(np.int64)

    # Delta_W[k, l] = s * sum_j c_j cos(2*pi*(k*u_j + l*v_j)/4096)
    s_fft = ALPHA / (D1 * D2)
    k_ix = np.arange(D1, dtype=np.int64)
    thU = ((k_ix[:, None] * u[None, :]) % D1) * (2.0 * np.pi / D1)
    thV = ((k_ix[:, None] * v[None, :]) % D2) * (2.0 * np.pi / D2)
    cs = (c32 * np.float32(s_fft))[None, :]
    delta = (np.cos(thU).astype(np.float32) * cs) @ np.cos(thV).astype(
        np.float32
    ).T - (np.sin(thU).astype(np.float32) * cs) @ np.sin(thV).astype(np.float32).T
    weff = W.T + delta  # [k, l]

    # quantization scales
    sx = np.float32(240.0 / np.abs(x2d).max())
    g = (240.0 / np.maximum(np.abs(weff).max(axis=0), 1e-20)).astype(
        np.float32
    )  # [l]  (full-column max: matches the validated error simulation)

    # fp8 weight part, blocked [lo, 128, NF8, 2, 256]
    w8 = np.clip(weff[:KF] * g[None, :], -240, 240).astype(F8)
    w8b = np.ascontiguousarray(
        w8.reshape(NF8, 2, 128, NLO, 256).transpose(3, 2, 0, 1, 4)
    )
    # bf16 weight part pre-scaled by sx*g, blocked [lo, cw, 128, q, 256]
    wb = (weff[KF:] * (sx * g)[None, :]).astype(BF)
    wbb = np.ascontiguousarray(
        wb.reshape(NWCH, KPCB, 128, NLO, 256).transpose(3, 2, 0, 1, 4)
    )
    bias_cols = np.ascontiguousarray(b.reshape(32, 128).T)
    scal_cols = np.ascontiguousarray(
        (1.0 / (sx * g)).astype(np.float32).reshape(32, 128).T
    )

    shared = {"w8b": w8b, "wbb": wbb, "biasc": bias_cols, "scalc": scal_cols}
    in_maps = []
    for core in range(NCORES):
        xs = x2d[core * S : (core + 1) * S, :]
        x8 = np.clip(xs[:, :KF].T * sx, -240, 240).astype(F8)
        x8c = np.ascontiguousarray(
            x8.reshape(NF8, 2, 128, S).transpose(2, 0, 1, 3)
        )
        xbt = xs[:, KF:].T.astype(BF)
        xbc = np.ascontiguousarray(
            xbt.reshape(NXCH, 2, 128, S).transpose(0, 2, 1, 3)
        )
        in_maps.append({"x8c": x8c, "xbc": xbc, **shared})
    return in_maps


def get_nc():
    if "nc" not in _CACHE:
        _CACHE["nc"] = _build_nc()
    return _CACHE["nc"]


def _axon_device_reset():
    """Best-effort recovery for a wedged axon terminal (NRT_EXEC_UNIT_...)."""
    try:
        import ctypes

        lib = ctypes.CDLL("/opt/axon/libaxon_pjrt.so")
        lib.axon_reset.restype = ctypes.c_int64
        import jax

        jax.devices()
        return lib.axon_reset() == 0
    except Exception:
        return False


def run(inputs, trace=False):
    nc = get_nc()
    in_maps = _host_prep(
        inputs["x"], inputs["c"], inputs["E"], inputs["W_base"], inputs["b_base"]
    )
    try:
        res = bass_utils.run_bass_kernel_spmd(
            nc, in_maps, core_ids=list(range(NCORES)), trace=trace
        )
    except Exception:
        if not _axon_device_reset():
            raise
        res = bass_utils.run_bass_kernel_spmd(
            nc, in_maps, core_ids=list(range(NCORES)), trace=trace
        )
    h = np.empty((S_TOTAL, D2), np.float32)
    for core in range(NCORES):
        h[core * S : (core + 1) * S, :] = res.results[core]["ht"].T.astype(
            np.float32
        )
    out = h.reshape(np.shape(inputs["x"])[:2] + (D2,))
    return out, res


def kernel(**inputs):
    out, _ = run(inputs)
    return out


# revision 30
# speedup vs baseline: 1.0081x; 1.0081x over previous
"""FourierFT fused kernel for Trainium2 (8 NeuronCores, SPMD data-parallel).

Computes h = x @ W_base^T + b_base + x @ Delta_W where
Delta_W = real(ifft2(scatter(c, E))) * ALPHA. The rank-200 Delta_W is folded
into the weight on the host (two [4096,100]x[100,4096] sgemms), so the device
runs a dense GEMM h^T = W_eff-contracted-with-x^T plus bias.

The GEMM is PE-bound (bf16 floor 437us/core; DMA measured 312 GB/s/core has
3x headroom), so the contraction is split by precision: the first KF=768 of
4096 k-values run as fp8(e4m3) DoubleRow matmuls (2 k-tiles per instruction,
2x throughput), the remaining 26 k-tiles in bf16. The fp8 fraction is capped
by accuracy: measured on the real inputs, 6/32 fp8 tiles give 1.6e-2 max rel
error vs the 2e-2 budget (8/32 would be 2.0e-2 - no margin).

Quantization: x8 = e4m3(x * sx), sx = 240/max|x| global; W8 col l scaled by
g[l] = 240/max|W_col|; the bf16 part of W is pre-scaled by sx*g[l] so both
parts accumulate in one PSUM in the same scaled units; the drain activation
applies the per-partition (per-l) scale 1/(sx*g[l]) and bias in one pass,
storing fp16 (halves output traffic; +6e-5 error).

Device layout: each core owns a 1024-row slice of x (flattened [8192,4096]),
pre-transposed k-major on the host. Output is produced as h^T tiles ([l,s]).
Schedule: the first ~30us is briefly DMA-bound (x + first weight chunks all
wanted early vs the 312 GB/s/core fabric cap), so the prefix issues pieces
in consumption order: bulk x and lo0 weight pieces interleaved on Sync
(ahead of main-loop weight prefetch so it cannot starve x), x8/bias/lo1
weights/outputs on Activation; weight chunks prefetch two output-chunks
ahead (DMA-complete semaphores propagate ~900ns, so JIT prefetch stalls).
Warmup matmuls ramp the PE clock before real data lands. lo0/lo1 are split
into two visits (fp8 + first weight chunk each, then second chunks), holding
all 8 PSUM banks, which defers 1.66MB of weight DMA out of the DMA-bound
prefix window. The last output chunk runs bank-major so three of its four
PSUM drains hide under matmuls. Steady state issues one matmul per 216ns
with zero gaps; measured 422-428us = 401us issue floor + ~7.5us framework
preamble (before any DMA can issue) + ~6us first-data ramp + ~11.5us fixed
framework teardown.
"""

import sys

if "/opt/trn_rl_repo" not in sys.path:
    sys.path.insert(0, "/opt/trn_rl_repo")

import numpy as np
import ml_dtypes

import concourse.bass as bass  # noqa: F401  (registers AP machinery)
import concourse.mybir as mybir
import concourse.tile as tile
from concourse import bacc, bass_utils

D1 = 4096
D2 = 4096
ALPHA = 300.0
NCORES = 8
S_TOTAL = 4 * 2048
S = S_TOTAL // NCORES  # 1024 rows per core
KT = D1 // 128  # 32 k-tiles
NLO = 16  # output column chunks of 256

NF8 = 3  # fp8 DoubleRow pairs per output chunk (2 k-tiles each)
KF = NF8 * 256  # k-values covered by fp8
KTB = KT - 2 * NF8  # bf16 k-tiles (26)
NWCH = 2  # bf16 weight chunks per lo
KPCB = KTB // NWCH  # 13 k-tiles per bf16 weight chunk
NXCH = KTB // 2  # bf16 x chunks (pairs of k-tiles)

F32 = mybir.dt.float32
F16 = mybir.dt.float16
BF16 = mybir.dt.bfloat16
FP8 = mybir.dt.float8e4
IDENT = mybir.ActivationFunctionType.Identity
DR = mybir.MatmulPerfMode.DoubleRow
BF = ml_dtypes.bfloat16
F8 = ml_dtypes.float8_e4m3

_CACHE = {}


def _build_nc():
    nc = bacc.Bacc("TRN2", target_bir_lowering=False, debug=False)

    x8_d = nc.dram_tensor("x8c", [128, NF8, 2, S], FP8, kind="ExternalInput").ap()
    xb_d = nc.dram_tensor("xbc", [NXCH, 128, 2, S], BF16, kind="ExternalInput").ap()
    w8_d = nc.dram_tensor(
        "w8b", [NLO, 128, NF8, 2, 256], FP8, kind="ExternalInput"
    ).ap()
    wb_d = nc.dram_tensor(
        "wbb", [NLO, 128, NWCH, KPCB, 256], BF16, kind="ExternalInput"
    ).ap()
    bias_d = nc.dram_tensor("biasc", [128, 32], F32, kind="ExternalInput").ap()
    scal_d = nc.dram_tensor("scalc", [128, 32], F32, kind="ExternalInput").ap()
    ht_d = nc.dram_tensor("ht", [D2, S], F16, kind="ExternalOutput").ap()

    with tile.TileContext(nc) as tc:
        with (
            tc.tile_pool(name="resident", bufs=1) as rpool,
            tc.tile_pool(name="wstream", bufs=9) as wpool,
            tc.tile_pool(name="wmerged", bufs=3) as wmpool,
            tc.tile_pool(name="outstage", bufs=3) as opool,
            tc.tile_pool(name="psum", bufs=8, space="PSUM") as ppool,
        ):
            x8_sb = rpool.tile([128, NF8, 2, S], FP8, tag="x8")
            xb_sb = rpool.tile([128, KTB, S], BF16, tag="xb")
            bias_sb = rpool.tile([128, 32], F32, tag="bias")
            scal_sb = rpool.tile([128, 32], F32, tag="scal")
            warm_sb = rpool.tile([128, 256], BF16, tag="warm")
            nc.gpsimd.memset(warm_sb[:], 0.0)

            def new_banks(lo):
                return [
                    [ppool.tile([128, 512], F32, tag="pm",
                                name=f"pms_{lo}_{j}_{h}")
                     for h in range(2)]
                    for j in range(2)
                ]

            def mm_f8(pms, w8t, p):
                for j in range(2):
                    lhsT = w8t[:, p, :, j * 128 : (j + 1) * 128]
                    for h in range(2):
                        nc.tensor.matmul(
                            pms[j][h],
                            lhsT,
                            x8_sb[:, p, :, h * 512 : (h + 1) * 512],
                            start=(p == 0),
                            stop=False,
                            perf_mode=DR,
                        )

            def mm_bf(pms, wbt, t):
                q = t % KPCB
                for j in range(2):
                    lhsT = wbt[:, q, j * 128 : (j + 1) * 128]
                    for h in range(2):
                        nc.tensor.matmul(
                            pms[j][h],
                            lhsT,
                            xb_sb[:, t, h * 512 : (h + 1) * 512],
                            start=False,
                            stop=(t == KTB - 1),
                        )

            def drain(pms, lo):
                for j in range(2):
                    lsub = lo * 2 + j
                    ot = opool.tile([128, S], F16, tag="ot")
                    for h in range(2):
                        nc.scalar.activation(
                            ot[:, h * 512 : (h + 1) * 512],
                            pms[j][h],
                            IDENT,
                            bias=bias_sb[:, lsub : lsub + 1],
                            scale=scal_sb[:, lsub : lsub + 1],
                        )
                    nc.scalar.dma_start(ht_d[lsub * 128 : (lsub + 1) * 128, :], ot)

            def load_w(lo):
                # two 832KB DMAs per lo, not one 1.66MB: per-DMA-engine
                # throughput is ~22.5 GB/s, so weight-stream rate comes from
                # DMA parallelism (a single merged DMA starves the PE, +35us)
                w8t = wpool.tile([128, NF8, 2, 256], FP8, tag="w8",
                                 name=f"w8_{lo}")
                nc.sync.dma_start(w8t, w8_d[lo])
                wbmt = wmpool.tile([128, NWCH, KPCB, 256], BF16, tag="wbm",
                                   name=f"wbm_{lo}")
                for cb in range(NWCH):
                    nc.sync.dma_start(wbmt[:, cb], wb_d[lo][:, cb])
                return w8t, [wbmt[:, 0], wbmt[:, 1]]

            # ---- prefix: consumption-ordered streams + PE clock warmup.
            # The first ~30us is briefly DMA-bound (x + lo0/lo1 weights all
            # wanted early), so pieces are issued on the Sync queue in the
            # order the PE consumes them; x8 goes on the Activation queue
            # in pair-sized pieces so the first DR matmul starts ASAP.
            for p in range(NF8):
                nc.scalar.dma_start(
                    x8_sb[:, p : p + 1, :, :], x8_d[:, p : p + 1, :, :]
                )
            nc.scalar.dma_start(bias_sb[:], bias_d[:])
            nc.scalar.dma_start(scal_sb[:], scal_d[:])
            w8t0 = wpool.tile([128, NF8, 2, 256], FP8, tag="w8", name="w8_0")
            nc.sync.dma_start(w8t0, w8_d[0])
            wbts0 = [
                wpool.tile([128, KPCB, 256], BF16, tag="wb", name=f"wb_0_{cb}")
                for cb in range(NWCH)
            ]

            def wb0_piece(cb, a, bnd):
                nc.sync.dma_start(
                    wbts0[cb][:, a:bnd, :], wb_d[0][:, cb, a:bnd, :]
                )

            def xb_chunk(c, eng):
                eng.dma_start(xb_sb[:, 2 * c : 2 * c + 2, :], xb_d[c])

            # lo1 weights ride the otherwise-idle Activation queue; bulk x
            # and lo0 weight pieces interleave by need-time on Sync, ahead
            # of the main-loop weight prefetch so it cannot starve x. The
            # second weight chunks (wb*[1]) queue after the x stream — they
            # are not consumed until the second visits at t~38/49us.
            w8t1 = wpool.tile([128, NF8, 2, 256], FP8, tag="w8", name="w8_1")
            nc.scalar.dma_start(w8t1, w8_d[1])
            wbts1 = [
                wpool.tile([128, KPCB, 256], BF16, tag="wb", name=f"wb_1_{cb}")
                for cb in range(NWCH)
            ]
            nc.scalar.dma_start(wbts1[0], wb_d[1][:, 0])
            wb0_piece(0, 0, 2)
            xb_chunk(0, nc.sync)
            wb0_piece(0, 2, 5)
            xb_chunk(1, nc.sync)
            wb0_piece(0, 5, 9)
            xb_chunk(2, nc.sync)
            wb0_piece(0, 9, 13)
            xb_chunk(3, nc.sync)
            xb_chunk(4, nc.sync)
            xb_chunk(5, nc.sync)
            w_tiles = {0: (w8t0, wbts0), 1: (w8t1, wbts1)}
            for c in range(6, NXCH):
                xb_chunk(c, nc.sync if c % 2 == 0 else nc.scalar)
            wb0_piece(1, 0, 13)
            nc.scalar.dma_start(wbts1[1], wb_d[1][:, 1])

            # warm the PE clock off warm_sb (memset lands ~7.2us, before any
            # DMA): enough back-to-back matmuls to bridge until real data
            pms0 = new_banks(0)
            for _ in range(14):
                nc.tensor.matmul(
                    pms0[0][0][:, 0:256],
                    warm_sb[:, 0:128],
                    warm_sb[:, 0:256],
                    start=True,
                    stop=False,
                    skip_group_check=True,
                )

            # ---- lo0/lo1 split into two visits: both run their fp8 pairs +
            # first bf16 weight chunk before either touches its second chunk,
            # deferring 1.66MB of weight DMA out of the DMA-bound prefix
            # window (the early PE gaps were exactly this stream deficit).
            # Holds both bank sets: 4 + 4 = all 8 PSUM banks.
            w8t0, wbts0 = w_tiles.pop(0)
            w8t1, wbts1 = w_tiles.pop(1)
            pms1 = new_banks(1)
            for p in range(NF8):
                mm_f8(pms0, w8t0, p)
            for t in range(KPCB):
                mm_bf(pms0, wbts0[0], t)
            for p in range(NF8):
                mm_f8(pms1, w8t1, p)
            for t in range(KPCB):
                mm_bf(pms1, wbts1[0], t)
            w_tiles[2] = load_w(2)
            for t in range(KPCB, KTB):
                mm_bf(pms0, wbts0[1], t)
            w_tiles[3] = load_w(3)
            for t in range(KPCB, KTB):
                mm_bf(pms1, wbts1[1], t)
            drain(pms0, 0)

            # ---- main loop (last chunk handled bank-major below)
            prev = pms1
            prev_lo = 1
            for lo in range(2, NLO - 1):
                for ahead in (lo + 1, lo + 2):
                    if ahead < NLO and ahead not in w_tiles:
                        w_tiles[ahead] = load_w(ahead)
                w8t, wbts = w_tiles.pop(lo)
                pms = new_banks(lo)
                for p in range(NF8):
                    mm_f8(pms, w8t, p)
                if prev is not None:
                    drain(prev, prev_lo)
                for t in range(KTB):
                    mm_bf(pms, wbts[t // KPCB], t)
                prev = pms
                prev_lo = lo

            # ---- last chunk: bank-major so three of the four PSUM drains
            # (and the first output DMA) hide under remaining matmuls
            lo = NLO - 1
            w8t, wbts = w_tiles.pop(lo)
            pms = new_banks(lo)
            ots = {}
            for idx, (j, h) in enumerate([(0, 0), (0, 1), (1, 0), (1, 1)]):
                for p in range(NF8):
                    nc.tensor.matmul(
                        pms[j][h],
                        w8t[:, p, :, j * 128 : (j + 1) * 128],
                        x8_sb[:, p, :, h * 512 : (h + 1) * 512],
                        start=(p == 0),
                        stop=False,
                        perf_mode=DR,
                    )
                if idx == 0:
                    drain(prev, prev_lo)
                for t in range(KTB):
                    q = t % KPCB
                    nc.tensor.matmul(
                        pms[j][h],
                        wbts[t // KPCB][:, q, j * 128 : (j + 1) * 128],
                        xb_sb[:, t, h * 512 : (h + 1) * 512],
                        start=False,
                        stop=(t == KTB - 1),
                    )
                if idx >= 1:
                    pj, ph = [(0, 0), (0, 1), (1, 0)][idx - 1]
                    lsub = lo * 2 + pj
                    if ph == 0:
                        ots[pj] = opool.tile([128, S], F16, tag="ot",
                                             name=f"otf{pj}")
                    nc.scalar.activation(
                        ots[pj][:, ph * 512 : (ph + 1) * 512],
                        pms[pj][ph],
                        IDENT,
                        bias=bias_sb[:, lsub : lsub + 1],
                        scale=scal_sb[:, lsub : lsub + 1],
                    )
                    if ph == 1:
                        nc.scalar.dma_start(
                            ht_d[lsub * 128 : (lsub + 1) * 128, :], ots[pj]
                        )
            lsub = lo * 2 + 1
            nc.scalar.activation(
                ots[1][:, 512:1024],
                pms[1][1],
                IDENT,
                bias=bias_sb[:, lsub : lsub + 1],
                scale=scal_sb[:, lsub : lsub + 1],
            )
            nc.scalar.dma_start(ht_d[lsub * 128 : (lsub + 1) * 128, :], ots[1])

    nc.compile()
    return nc


def _host_prep(x, c, E, W_base, b_base):
    """Fold Delta_W into W, quantize, shard + lay out inputs."""
    x2d = np.ascontiguousarray(
        np.asarray(x, dtype=np.float32).reshape(S_TOTAL, D1)
    )
    W = np.asarray(W_base, dtype=np.float32)
    b = np.asarray(b_base, dtype=np.float32)
    c32 = np.asarray(c, dtype=np.float32)
    u = np.asarray(E[0]).astype(np.int64)
    v = np.asarray(E[1]).astype(np.int64)

    # Delta_W[k, l] = s * sum_j c_j cos(2*pi*(k*u_j + l*v_j)/4096)
    s_fft = ALPHA / (D1 * D2)
    k_ix = np.arange(D1, dtype=np.int64)
    thU = ((k_ix[:, None] * u[None, :]) % D1) * (2.0 * np.pi / D1)
    thV = ((k_ix[:, None] * v[None, :]) % D2) * (2.0 * np.pi / D2)
    cs = (c32 * np.float32(s_fft))[None, :]
    delta = (np.cos(thU).astype(np.float32) * cs) @ np.cos(thV).astype(
        np.float32
    ).T - (np.sin(thU).astype(np.float32) * cs) @ np.sin(thV).astype(np.float32).T
    weff = W.T + delta  # [k, l]

    # quantization scales
    sx = np.float32(240.0 / np.abs(x2d).max())
    g = (240.0 / np.maximum(np.abs(weff).max(axis=0), 1e-20)).astype(
        np.float32
    )  # [l]  (full-column max: matches the validated error simulation)

    # fp8 weight part, blocked [lo, 128, NF8, 2, 256]
    w8 = np.clip(weff[:KF] * g[None, :], -240, 240).astype(F8)
    w8b = np.ascontiguousarray(
        w8.reshape(NF8, 2, 128, NLO, 256).transpose(3, 2, 0, 1, 4)
    )
    # bf16 weight part pre-scaled by sx*g, blocked [lo, cw, 128, q, 256]
    wb = (weff[KF:] * (sx * g)[None, :]).astype(BF)
    wbb = np.ascontiguousarray(
        wb.reshape(NWCH, KPCB, 128, NLO, 256).transpose(3, 2, 0, 1, 4)
    )
    bias_cols = np.ascontiguousarray(b.reshape(32, 128).T)
    scal_cols = np.ascontiguousarray(
        (1.0 / (sx * g)).astype(np.float32).reshape(32, 128).T
    )

    shared = {"w8b": w8b, "wbb": wbb, "biasc": bias_cols, "scalc": scal_cols}
    in_maps = []
    for core in range(NCORES):
        xs = x2d[core * S : (core + 1) * S, :]
        x8 = np.clip(xs[:, :KF].T * sx, -240, 240).astype(F8)
        x8c = np.ascontiguousarray(
            x8.reshape(NF8, 2, 128, S).transpose(2, 0, 1, 3)
        )
        xbt = xs[:, KF:].T.astype(BF)
        xbc = np.ascontiguousarray(
            xbt.reshape(NXCH, 2, 128, S).transpose(0, 2, 1, 3)
        )
        in_maps.append({"x8c": x8c, "xbc": xbc, **shared})
    return in_maps


def get_nc():
    if "nc" not in _CACHE:
        _CACHE["nc"] = _build_nc()
    return _CACHE["nc"]


def _axon_device_reset():
    """Best-effort recovery for a wedged axon terminal (NRT_EXEC_UNIT_...)."""
    try:
        import ctypes

        lib = ctypes.CDLL("/opt/axon/libaxon_pjrt.so")
        lib.axon_reset.restype = ctypes.c_int64
        import jax

        jax.devices()
        return lib.axon_reset() == 0
    except Exception:
        return False


def run(inputs, trace=False):
    nc = get_nc()
    in_maps = _host_prep(
        inputs["x"], inputs["c"], inputs["E"], inputs["W_base"], inputs["b_base"]
    )
    try:
        res = bass_utils.run_bass_kernel_spmd(
            nc, in_maps, core_ids=list(range(NCORES)), trace=trace
        )
    except Exception:
        if not _axon_device_reset():
            raise
        res = bass_utils.run_bass_kernel_spmd(
            nc, in_maps, core_ids=list(range(NCORES)), trace=trace
        )
    h = np.empty((S_TOTAL, D2), np.float32)
    for core in range(NCORES):
        h[core * S : (core + 1) * S, :] = res.results[core]["ht"].T.astype(
            np.float32
        )
    out = h.reshape(np.shape(inputs["x"])[:2] + (D2,))
    return out, res


def kernel(**inputs):
    out, _ = run(inputs)
    return out
